# revision 1
# baseline (speedup 1.0000x reference)
"""GAT-style message passing kernel for Trainium2 (8 NeuronCores, data-parallel over nodes).

Reference computation (per node n, K=16 neighbors, D=DOUT=128):
    neigh_self = concat([neigh_vecs[n], self_vecs[n][None]], 0)      # [17, 128]
    score      = neigh_self @ self_vecs[n]                           # [17]
    attn       = softmax(score)
    ctx        = attn @ neigh_self                                   # [128]
    out[n]     = relu(ctx @ W)                                       # [128]

Sharding: rows (nodes) split evenly across 8 cores, weights replicated.
Per-core kernel (all fp32): stream 128-node tiles; per tile
  - DVE scalar_tensor_tensor (fused mul + free-dim-sum) x17 -> scores
  - softmax via reduce_max(negate)/ACT exp(+accum sum)/reciprocal
  - DVE chained scalar_tensor_tensor (fused mul-add) x17 -> ctx (unnormalized)
  - PE transpose(ctx) -> PSUM, ACT copy -> SBUF, PE matmul with W
  - ACT relu with scale=1/sum_exp (folds softmax normalization into the relu)

Measured (8-core SPMD, per-execution, repeat-delta method): ~590-810us
(median ~650us); memory roofline for the 922MB of traffic is ~320us/core.
The kernel is DVE-bound: 34 x ~190ns fused ops per 128-node tile.
Variants measured and rejected: bf16 passes (STT gets no 2x mode on HW; the
fp32->bf16 convert costs more than it saves), PE-built context via
diag(e_k) matmuls (poor cross-engine overlap), split DMA / deeper buffers
(no change - DMA is not the bottleneck).
"""

import sys

if "/opt/trn_rl_repo" not in sys.path:
    sys.path.insert(0, "/opt/trn_rl_repo")

import numpy as np

N, K, D = 100000, 16, 128
NCORES = 8
TILE_P = 128
# nodes per core, padded up so every core gets a whole number of 128-row tiles
NC_NODES = ((N + NCORES * TILE_P - 1) // (NCORES * TILE_P)) * TILE_P  # 12544
NTILES = NC_NODES // TILE_P  # 98

# kernel variant knobs (chosen by benchmarking; see bench_variants.py)
BF16_SCORES = False
BF16_CTX = False
CONVERT_ENGINE = "gpsimd"  # "gpsimd" | "act" | "vector"

# best measured config (HW A/B: hw_ab2.py repeat-delta; hw_head2head.py same-
# session A/B). v1-fp32: 593-653us | +act_offload: +16us vs v1 (same-session) |
# pipelined emission: 701us | hybrid pe_split J=11: 709us | pe_ctx diag-matmuls:
# 780us | bf16(act cvt): 699us | bf16(gpsimd cvt): 1435us
BEST = dict(bf16_scores=False, bf16_ctx=False, pe_ctx=False, act_offload=False)

_cached_nc = None


def _build(
    nc_nodes=NC_NODES,
    bf16_scores=BF16_SCORES,
    bf16_ctx=BF16_CTX,
    convert_engine=CONVERT_ENGINE,
    bufs=None,
    ablate=(),
    repeat=1,
    split_dma=False,
    pe_ctx=False,
    diag_dve=5,
    pe_split=0,
    act_offload=False,
    pipelined=False,
):
    import concourse.mybir as mybir
    import concourse.tile as tile
    from concourse import bacc
    from concourse.masks import make_identity

    f32 = mybir.dt.float32
    bf16 = mybir.dt.bfloat16
    Alu = mybir.AluOpType
    Act = mybir.ActivationFunctionType
    ntiles = nc_nodes // TILE_P
    use_bf16 = bf16_scores or bf16_ctx
    b = dict(ns=4, work=2, accp=2, outp=3, psum=2)
    if bufs:
        b.update(bufs)

    nc = bacc.Bacc("TRN2", debug=False)
    sv = nc.dram_tensor("self_vecs", (nc_nodes, D), f32, kind="ExternalInput").ap()
    gv = nc.dram_tensor("neigh_vecs", (nc_nodes, K, D), f32, kind="ExternalInput").ap()
    wt = nc.dram_tensor("weights", (D, D), f32, kind="ExternalInput").ap()
    out = nc.dram_tensor("out", (nc_nodes, D), f32, kind="ExternalOutput").ap()

    with tile.TileContext(nc) as tc:
        with (
            tc.tile_pool(name="singles", bufs=1) as singles,
            tc.tile_pool(name="ns", bufs=b["ns"]) as nsp,
            tc.tile_pool(name="work", bufs=b["work"]) as wp,
            tc.tile_pool(name="accp", bufs=b["accp"]) as accp,
            tc.tile_pool(name="outp", bufs=b["outp"]) as outp,
            tc.tile_pool(name="psum", bufs=b["psum"], space="PSUM") as pp,
        ):
            w_sb = singles.tile([D, D], f32)
            nc.sync.dma_start(out=w_sb, in_=wt)
            mm_dt = bf16 if bf16_ctx else f32
            if bf16_ctx:
                w_mm = singles.tile([D, D], bf16)
                nc.scalar.copy(w_mm, w_sb)
            else:
                w_mm = w_sb
            ident = singles.tile([TILE_P, TILE_P], mm_dt)
            make_identity(nc, ident)

            if pipelined:
                # software-pipelined emission: DVE runs tile t's ctx chain
                # while ACT computes tile t+1's exp, so the DVE never waits
                # on the softmax round-trip.
                state = {}

                def s_load(t):
                    r0 = (t % ntiles) * TILE_P
                    ns_ = nsp.tile([TILE_P, K + 1, D], f32, tag="ns")
                    nc.sync.dma_start(
                        out=ns_[:, 0:K, :], in_=gv[r0 : r0 + TILE_P, :, :]
                    )
                    nc.sync.dma_start(out=ns_[:, K, :], in_=sv[r0 : r0 + TILE_P, :])
                    state[t] = {"ns": ns_}

                def s_scores(t):
                    st = state[t]
                    ns_ = st["ns"]
                    selfv_ = ns_[:, K, :]
                    scores = wp.tile([TILE_P, K + 1], f32, tag="scores")
                    prod = wp.tile([TILE_P, D], f32, tag="prod")
                    for k in range(K):
                        nc.vector.scalar_tensor_tensor(
                            out=prod,
                            in0=ns_[:, k, :],
                            scalar=1.0,
                            in1=selfv_,
                            op0=Alu.mult,
                            op1=Alu.mult,
                            accum_out=scores[:, k : k + 1],
                        )
                    prod2 = wp.tile([TILE_P, D], f32, tag="prod2")
                    nc.scalar.activation(
                        prod2, selfv_, Act.Square, accum_out=scores[:, K : K + 1]
                    )
                    negmax = wp.tile([TILE_P, 1], f32, tag="negmax")
                    nc.vector.tensor_reduce(
                        negmax, scores, mybir.AxisListType.X, Alu.max, negate=True
                    )
                    e_ = wp.tile([TILE_P, K + 1], f32, tag="e")
                    sumexp = wp.tile([TILE_P, 1], f32, tag="sumexp")
                    nc.scalar.activation(
                        e_, scores, Act.Exp, bias=negmax, scale=1.0, accum_out=sumexp
                    )
                    st["e"] = e_
                    st["sumexp"] = sumexp

                def s_ctx_tail(t):
                    st = state.pop(t)
                    ns_, e_ = st["ns"], st["e"]
                    r0 = (t % ntiles) * TILE_P
                    inv_ = wp.tile([TILE_P, 1], f32, tag="inv")
                    nc.vector.reciprocal(inv_, st["sumexp"])
                    acc = accp.tile([TILE_P, D], f32, tag="acc")
                    nc.scalar.mul(acc, ns_[:, 0, :], e_[:, 0:1])
                    for k in range(1, K + 1):
                        acc2 = accp.tile([TILE_P, D], f32, tag="acc")
                        nc.vector.scalar_tensor_tensor(
                            out=acc2,
                            in0=ns_[:, k, :],
                            scalar=e_[:, k : k + 1],
                            in1=acc,
                            op0=Alu.mult,
                            op1=Alu.add,
                        )
                        acc = acc2
                    ctxT_ps = pp.tile([TILE_P, TILE_P], f32, tag="ctxT")
                    nc.tensor.transpose(ctxT_ps, acc, ident)
                    ctxT = wp.tile([TILE_P, TILE_P], f32, tag="ctxT_sb")
                    nc.scalar.copy(ctxT, ctxT_ps)
                    out_ps = pp.tile([TILE_P, TILE_P], f32, tag="out_ps")
                    nc.tensor.matmul(
                        out_ps, lhsT=ctxT, rhs=w_mm, start=True, stop=True
                    )
                    ob = outp.tile([TILE_P, D], f32, tag="ob")
                    nc.scalar.activation(ob, out_ps, Act.Relu, bias=0.0, scale=inv_)
                    nc.sync.dma_start(out=out[r0 : r0 + TILE_P, :], in_=ob)

                total = ntiles * repeat
                for i in range(total + 2):
                    if i < total:
                        s_load(i)
                    if 1 <= i <= total:
                        s_scores(i - 1)
                    if i >= 2:
                        s_ctx_tail(i - 2)

            for t in range(0 if pipelined else ntiles * repeat):
                t = t % ntiles
                r0 = t * TILE_P
                # [128 nodes(part), 17 keys, 128 d]; key 16 is the self vector
                ns = nsp.tile([TILE_P, K + 1, D], f32, tag="ns")
                if split_dma:
                    nc.sync.dma_start(
                        out=ns[:, 0 : K // 2, :], in_=gv[r0 : r0 + TILE_P, 0 : K // 2, :]
                    )
                    nc.scalar.dma_start(
                        out=ns[:, K // 2 : K, :], in_=gv[r0 : r0 + TILE_P, K // 2 : K, :]
                    )
                else:
                    nc.sync.dma_start(
                        out=ns[:, 0:K, :], in_=gv[r0 : r0 + TILE_P, :, :]
                    )
                nc.sync.dma_start(out=ns[:, K, :], in_=sv[r0 : r0 + TILE_P, :])

                if use_bf16:
                    nsb = nsp.tile([TILE_P, K + 1, D], bf16, tag="nsb")
                    cv = {
                        "gpsimd": nc.gpsimd,
                        "act": nc.scalar,
                        "vector": nc.vector,
                    }[convert_engine]
                    if convert_engine == "act":
                        cv.copy(nsb, ns)
                    else:
                        cv.tensor_copy(nsb, ns)
                else:
                    nsb = ns

                s_src = nsb if bf16_scores else ns
                s_dt = bf16 if bf16_scores else f32
                selfv = s_src[:, K, :]

                # scores[n, k] = sum_d ns[n,k,d] * self[n,d]
                scores = wp.tile([TILE_P, K + 1], f32, tag="scores")
                if "scores" in ablate:
                    nc.vector.memset(scores, 0.0)
                else:
                    prod = wp.tile([TILE_P, D], s_dt, tag="prod")
                    n_dve_scores = K if act_offload else K + 1
                    for k in range(n_dve_scores):
                        nc.vector.scalar_tensor_tensor(
                            out=prod,
                            in0=s_src[:, k, :],
                            scalar=1.0,
                            in1=selfv,
                            op0=Alu.mult,
                            op1=Alu.mult,
                            accum_out=scores[:, k : k + 1],
                        )
                    if act_offload:
                        # self-score = sum(self^2) on the scalar engine
                        prod2 = wp.tile([TILE_P, D], f32, tag="prod2")
                        nc.scalar.activation(
                            prod2,
                            selfv,
                            Act.Square,
                            accum_out=scores[:, K : K + 1],
                        )

                # softmax pieces: e = exp(s - max), sum_e; normalization folded
                # into the final relu's scale
                e = wp.tile([TILE_P, K + 1], f32, tag="e")
                inv = wp.tile([TILE_P, 1], f32, tag="inv")
                if "softmax" in ablate:
                    nc.vector.memset(e, 0.05)
                    nc.vector.memset(inv, 1.0)
                else:
                    negmax = wp.tile([TILE_P, 1], f32, tag="negmax")
                    nc.vector.tensor_reduce(
                        negmax, scores, mybir.AxisListType.X, Alu.max, negate=True
                    )
                    sumexp = wp.tile([TILE_P, 1], f32, tag="sumexp")
                    nc.scalar.activation(
                        e, scores, Act.Exp, bias=negmax, scale=1.0, accum_out=sumexp
                    )
                    nc.vector.reciprocal(inv, sumexp)

                c_src = nsb if bf16_ctx else ns
                c_dt = bf16 if bf16_ctx else f32
                if pe_ctx:
                    # ctxT[d,n] = sum_k ns[n,k,d]*e[n,k] via PE: for each k,
                    # matmul(lhsT=ns_k [n,d], rhs=diag(e_k) [n,n]) accumulated
                    # in PSUM. diag(e_k) = identity * e[:,k] (per-row scale).
                    ctxT_ps = pp.tile([TILE_P, TILE_P], f32, tag="ctxT")
                    for k in range(K + 1):
                        dg = accp.tile([TILE_P, TILE_P], c_dt, tag="diag")
                        if k < diag_dve:
                            nc.vector.tensor_scalar_mul(dg, ident, e[:, k : k + 1])
                        else:
                            nc.scalar.mul(dg, ident, e[:, k : k + 1])
                        nc.tensor.matmul(
                            ctxT_ps,
                            lhsT=c_src[:, k, :],
                            rhs=dg,
                            start=(k == 0),
                            stop=(k == K),
                        )
                else:
                    # ctx[n,d] via chained fused mul-add on DVE, for keys
                    # [0, n_chain); keys [n_chain, K] go through PE diag
                    # matmuls (diags built on ACT) accumulated in PSUM,
                    # merged with the DVE part via a transpose-mode matmul.
                    n_chain = pe_split if pe_split else K + 1
                    acc = accp.tile([TILE_P, D], c_dt, tag="acc")
                    if act_offload:
                        nc.scalar.mul(acc, c_src[:, 0, :], e[:, 0:1])
                    else:
                        nc.vector.tensor_scalar_mul(acc, c_src[:, 0, :], e[:, 0:1])
                    if "ctx" not in ablate:
                        for k in range(1, n_chain):
                            acc2 = accp.tile([TILE_P, D], c_dt, tag="acc")
                            nc.vector.scalar_tensor_tensor(
                                out=acc2,
                                in0=c_src[:, k, :],
                                scalar=e[:, k : k + 1],
                                in1=acc,
                                op0=Alu.mult,
                                op1=Alu.add,
                            )
                            acc = acc2
                    if pe_split:
                        ctxT_ps = pp.tile([TILE_P, TILE_P], f32, tag="ctxT")
                        for idx, k in enumerate(range(n_chain, K + 1)):
                            dg = accp.tile([TILE_P, TILE_P], f32, tag="diag")
                            nc.scalar.mul(dg, ident, e[:, k : k + 1])
                            nc.tensor.matmul(
                                ctxT_ps,
                                lhsT=c_src[:, k, :],
                                rhs=dg,
                                start=(idx == 0),
                                stop=False,
                            )
                        nc.tensor.matmul(
                            ctxT_ps,
                            lhsT=acc,
                            rhs=ident,
                            is_transpose=True,
                            start=False,
                            stop=True,
                        )

                # out = relu((ctx @ W) / sum_e): transpose ctx so d is on
                # partitions, then PE matmul with W, then ACT relu+scale
                ob = outp.tile([TILE_P, D], f32, tag="ob")
                if "tail" in ablate:
                    nc.scalar.activation(ob, acc, Act.Relu, bias=0.0, scale=inv)
                elif pe_ctx or pe_split:
                    ctxT = wp.tile([TILE_P, TILE_P], mm_dt, tag="ctxT_sb")
                    nc.scalar.copy(ctxT, ctxT_ps)
                    out_ps = pp.tile([TILE_P, TILE_P], f32, tag="out_ps")
                    nc.tensor.matmul(
                        out_ps, lhsT=ctxT, rhs=w_mm, start=True, stop=True
                    )
                    nc.scalar.activation(ob, out_ps, Act.Relu, bias=0.0, scale=inv)
                else:
                    ctxT_ps = pp.tile([TILE_P, TILE_P], mm_dt, tag="ctxT")
                    nc.tensor.transpose(ctxT_ps, acc, ident)
                    ctxT = wp.tile([TILE_P, TILE_P], mm_dt, tag="ctxT_sb")
                    nc.scalar.copy(ctxT, ctxT_ps)
                    out_ps = pp.tile([TILE_P, TILE_P], f32, tag="out_ps")
                    nc.tensor.matmul(
                        out_ps, lhsT=ctxT, rhs=w_mm, start=True, stop=True
                    )
                    nc.scalar.activation(ob, out_ps, Act.Relu, bias=0.0, scale=inv)
                nc.sync.dma_start(out=out[r0 : r0 + TILE_P, :], in_=ob)

    nc.compile()
    return nc


def _get_nc():
    global _cached_nc
    if _cached_nc is None:
        _cached_nc = _build(**BEST)
    return _cached_nc


def run_sharded(self_vecs, neigh_vecs, weights, trace=False, nc=None):
    """Shard inputs over 8 cores, run, gather. Returns (out, BassKernelResults)."""
    from concourse import bass_utils

    self_vecs = np.asarray(self_vecs, dtype=np.float32)
    neigh_vecs = np.asarray(neigh_vecs, dtype=np.float32)
    weights = np.asarray(weights, dtype=np.float32)

    n = self_vecs.shape[0]
    total = NCORES * NC_NODES
    pad = total - n
    if pad:
        self_p = np.concatenate(
            [self_vecs, np.zeros((pad, D), np.float32)], axis=0
        )
        neigh_p = np.concatenate(
            [neigh_vecs, np.zeros((pad, K, D), np.float32)], axis=0
        )
    else:
        self_p, neigh_p = self_vecs, neigh_vecs

    in_maps = []
    for c in range(NCORES):
        lo, hi = c * NC_NODES, (c + 1) * NC_NODES
        in_maps.append(
            {
                "self_vecs": np.ascontiguousarray(self_p[lo:hi]),
                "neigh_vecs": np.ascontiguousarray(neigh_p[lo:hi]),
                "weights": weights,
            }
        )

    if nc is None:
        nc = _get_nc()
    try:
        res = bass_utils.run_bass_kernel_spmd(
            nc, in_maps, core_ids=list(range(NCORES)), trace=trace
        )
    except ModuleNotFoundError:
        # NTFF profiling hook unavailable in this container; run untraced
        import os

        os.environ["BASS_NEVER_TRACE"] = "1"
        res = bass_utils.run_bass_kernel_spmd(
            nc, in_maps, core_ids=list(range(NCORES)), trace=False
        )
    out = np.concatenate([res.results[c]["out"] for c in range(NCORES)], axis=0)[:n]
    return out, res


def kernel(self_vecs, neigh_vecs, weights):
    out, _ = run_sharded(self_vecs, neigh_vecs, weights, trace=False)
    return out

